# revision 1
# baseline (speedup 1.0000x reference)
"""BitNet decoder MLP on 8 Trainium2 NeuronCores (Bass/Tile).

Strategy: data-parallel over batch (512 rows/core). Weights are ternary-quantized
cooperatively (each core quantizes 1/8 of all weight chunks; per-layer
AllReduce for the |W| mean, per-layer AllGather of the quantized bf16 image so
layer 0's weights are available early). All matmul arithmetic is exact:
activations are int8-valued bf16, weights are {-1,0,1} bf16, accumulation fp32
in PSUM. Per-row dequant scales fold into the PSUM-eviction pass on the scalar
engine; LayerNorm+SiLU run as fused scalar-engine passes (sigmoid via the tanh
table for accuracy); rsqrt via Newton on the vector engine; rounding via the
fp32 magic-number trick (round-half-even, matches jnp.round).

Weight images are stored unit-major ("unit" = one weight panel, a
[128, panel_ic*512] block with contiguous per-partition rows) so every big DMA
moves 16KB-contiguous per-partition descriptors.
"""

import numpy as np

import concourse.bass as bass
import concourse.mybir as mybir
import concourse.tile as tile
from concourse import bacc
from concourse.bass_utils import run_bass_kernel_spmd

F32 = mybir.dt.float32
BF16 = mybir.dt.bfloat16
AF = mybir.ActivationFunctionType
OP = mybir.AluOpType

N_CORES = 8
P = 128
OBW = 512            # output block width (one PSUM bank of fp32)
CH_ELS = P * OBW     # elements per weight chunk
MAGIC = 12582912.0   # 1.5 * 2**23: fp32 round-to-nearest-even trick
EPS = 1e-5

FULL_CFG = dict(B=4096, D0=1024, H=4096, OBINS=1000)


def _plan(cfg):
    """Static per-layer plan."""
    B, D0, H, OBINS = cfg["B"], cfg["D0"], cfg["H"], cfg["OBINS"]
    o3_real = 2 * OBINS
    o3_pad = ((o3_real + OBW - 1) // OBW) * OBW
    dims = [
        dict(din=D0, dout=H, dreal=H),
        dict(din=H, dout=H, dreal=H),
        dict(din=H, dout=H, dreal=H),
        dict(din=H, dout=o3_pad, dreal=o3_real),
    ]
    numels = [H * D0, H * H, H * H, o3_real * H]  # real numels for mean|W|
    layers = []
    ch_base = 0
    for li, d in enumerate(dims):
        n_ic = d["din"] // P
        n_ob = d["dout"] // OBW
        n_ch = n_ob * n_ic
        assert n_ch % N_CORES == 0, (li, n_ch)
        panel_ic = min(16, n_ic, max(1, n_ch // N_CORES))
        assert n_ic % panel_ic == 0 and (n_ch // N_CORES) % panel_ic == 0
        n_panels = n_ic // panel_ic
        layers.append(dict(
            li=li, din=d["din"], dout=d["dout"], dreal=d["dreal"],
            n_ic=n_ic, n_ob=n_ob, n_ch=n_ch, per_rank=n_ch // N_CORES,
            panel_ic=panel_ic, n_panels=n_panels,
            numel=numels[li], ch_base=ch_base,
            n_halves=2 if (n_ch // N_CORES) % 2 == 0 and (n_ch // N_CORES) // 2 % panel_ic == 0 else 1,
            ob_w=[min(OBW, d["dreal"] - ob * OBW) for ob in range(n_ob)],
        ))
        ch_base += n_ch
    total_ch = ch_base
    per_rank = total_ch // N_CORES
    b_core = B // N_CORES
    assert b_core % P == 0
    return layers, total_ch, per_rank, b_core // P


def _rsqrt_newton(nc, pool, v, n_iter=3):
    """istd = 1/sqrt(v) for v [128,1] fp32 (v > 0), pure-DVE Newton iteration.

    seed_bits = 0x5f370000 - bits(v)/2 computed in fp32 on aligned int32 views;
    the fp32 mantissa noise on the >2^24 intermediate is irrelevant for a seed.
    """
    seed = pool.tile([P, 1], F32, tag="rs_seed", name="rs_seed")
    seed_i32 = seed[:].bitcast(mybir.dt.int32)
    v_i32 = v.bitcast(mybir.dt.int32)
    nc.vector.tensor_scalar(seed_i32[:], v_i32[:], -0.5,
                            float(0x5F370000), OP.mult, OP.add)
    y = seed
    t1 = pool.tile([P, 1], F32, tag="rs_t1", name="rs_t1")
    t2 = pool.tile([P, 1], F32, tag="rs_t2", name="rs_t2")
    for _ in range(n_iter):
        nc.vector.tensor_tensor(t1[:], y[:], y[:], OP.mult)
        nc.vector.tensor_tensor(t2[:], t1[:], v, OP.mult)
        nc.vector.tensor_scalar(t1[:], t2[:], -0.5, 1.5, OP.mult, OP.add)
        nc.vector.tensor_tensor(y[:], y[:], t1[:], OP.mult)
    return y


def build(cfg):
    layers, total_ch, per_rank, T = _plan(cfg)
    nc = bacc.Bacc("TRN2", target_bir_lowering=False, debug=False,
                   num_devices=N_CORES)

    D0, OBINS = cfg["D0"], cfg["OBINS"]
    b_core = T * P

    xs = nc.dram_tensor("xs", [b_core, D0], F32, kind="ExternalInput")
    # rank's weight chunks, unit-major flat fp32 (see prepare_inputs)
    wsh = nc.dram_tensor("wsh", [per_rank * CH_ELS], F32, kind="ExternalInput")
    mz_out = nc.dram_tensor("mz", [b_core, OBINS], F32, kind="ExternalOutput")
    ii_out = nc.dram_tensor("ii", [b_core, OBINS], F32, kind="ExternalOutput")

    with tile.TileContext(nc) as tc:
        with (
            tc.tile_pool(name="ybig", bufs=4) as ypool,        # 16KB/partition f32
            tc.tile_pool(name="wr", bufs=3) as wrpool,         # prep fp32 runs 8KB
            tc.tile_pool(name="xqT", bufs=5) as xqTpool,       # [128,32,128] bf16
            tc.tile_pool(name="xqT0", bufs=4) as xqT0pool,     # [128,n_ic0,128] bf16
            tc.tile_pool(name="wp", bufs=2) as wpool,          # [128,16,512] bf16
            tc.tile_pool(name="xqn", bufs=2) as xqnpool,       # 8KB/partition bf16
            tc.tile_pool(name="sg", bufs=2) as sgpool,         # [128,512] f32
            tc.tile_pool(name="u", bufs=2) as upool,           # [128,512] f32
            tc.tile_pool(name="outr", bufs=2) as outpool,      # [128,OBINS] f32
            tc.tile_pool(name="small", bufs=1) as small,
            tc.tile_pool(name="psum", bufs=8, space="PSUM") as psum,
            tc.tile_pool(name="dram", bufs=1, space="DRAM") as dram,
        ):
            # ---------------- DRAM scratch (flat, unit-major) ----------------
            stage = []
            image = []
            for L in layers:
                nh = L["n_halves"]
                hs = L["per_rank"] // nh * CH_ELS
                hi = L["n_ch"] // nh * CH_ELS
                stage.append([dram.tile([hs], BF16, tag=f"stage{L['li']}_{h}",
                                        name=f"stage{L['li']}_{h}")
                              for h in range(nh)])
                image.append([dram.tile([hi], BF16, tag=f"image{L['li']}_{h}",
                                        name=f"image{L['li']}_{h}",
                                        addr_space="Shared")
                              for h in range(nh)])
            ar_in = [dram.tile([P, 1], F32, tag=f"ar_in{l}", name=f"ar_in{l}")
                     for l in range(1)]
            ar_out = [dram.tile([P, 1], F32, tag=f"ar_out{l}",
                                name=f"ar_out{l}", addr_space="Shared")
                      for l in range(1)]
            ar_in123 = dram.tile([P, 3], F32, tag="ar_in123", name="ar_in123")
            ar_out123 = dram.tile([P, 3], F32, tag="ar_out123",
                                  name="ar_out123", addr_space="Shared")

            # ---------------- Stage A: input activation quant ----------------
            n_ic0 = layers[0]["n_ic"]
            xqT_cur = []
            am0s = []
            for t in range(T):
                xt = ypool.tile([P, D0], F32, tag="y", name=f"xt{t}")
                nc.sync.dma_start(xt[:], xs[t * P:(t + 1) * P, :])
                am = small.tile([P, 1], F32, tag=f"am0_{t}", name=f"am0_{t}")
                nc.vector.tensor_reduce(am[:], xt[:], mybir.AxisListType.X,
                                        OP.max, apply_absolute_value=True)
                nc.vector.tensor_scalar(am[:], am[:], float(EPS), None, OP.max)
                sc = small.tile([P, 1], F32, tag=f"s0_{t}", name=f"s0_{t}")
                nc.vector.tensor_scalar(sc[:], am[:], 1.0 / 127.0, None, OP.mult)
                nc.vector.reciprocal(sc[:], sc[:])
                xq0 = xqnpool.tile([P, D0], BF16, tag="xqn", name=f"xq0_{t}")
                for ch in range(D0 // OBW):
                    uu = upool.tile([P, OBW], F32, tag="u", name=f"u0_{t}_{ch}")
                    nc.scalar.activation(uu[:], xt[:, ch * OBW:(ch + 1) * OBW],
                                         AF.Copy, bias=MAGIC, scale=sc[:])
                    nc.vector.tensor_scalar(xq0[:, ch * OBW:(ch + 1) * OBW],
                                            uu[:], MAGIC, None, OP.subtract)
                xqT0 = xqT0pool.tile([P, n_ic0, P], BF16, tag="xqT0",
                                     name=f"xqT0_{t}")
                nc.scalar.dma_start_transpose(xqT0[:], xq0[:])
                xqT_cur.append(xqT0)
                am0s.append(am)

            # ------- Stages B/C: scale ARs, quantize, AllGathers -----
            # Critical path: L0 fully first (tiny), then one partial pass +
            # one AllReduce for L1-3, then per-layer quantize + AllGather.
            RUN = 4
            layer_jofs = {}
            jofs = 0
            for L in layers:
                layer_jofs[L["li"]] = jofs
                jofs += L["per_rank"]

            def _abs_pass(li, partial_col, partials):
                L = layers[li]
                pr, jofs = L["per_rank"], layer_jofs[li]
                nrun = 0
                for h in range(0, pr, RUN):
                    rl = min(RUN, pr - h)
                    off = (jofs + h) * CH_ELS
                    wrun = wrpool.tile([P, rl * OBW], F32, tag="wr",
                                       name=f"wrB{li}_{h}")
                    nc.sync.dma_start(
                        wrun[:], wsh[off:off + rl * CH_ELS].rearrange(
                            "(p f) -> p f", p=P))
                    nc.scalar.activation(wrun[:], wrun[:], AF.Abs,
                                         bias=0.0, scale=1.0,
                                         accum_out=partials[:, partial_col + nrun:
                                                            partial_col + nrun + 1])
                    nrun += 1
                return nrun

            def _quant_layer(li, swl):
                L = layers[li]
                pr, jofs = L["per_rank"], layer_jofs[li]
                nh = L["n_halves"]
                prh = pr // nh
                for half in range(nh):
                    for h in range(0, prh, RUN):
                        rl = min(RUN, prh - h)
                        hh = half * prh + h
                        off = (jofs + hh) * CH_ELS
                        wrun = wrpool.tile([P, rl * OBW], F32, tag="wr",
                                           name=f"wrC{li}_{hh}")
                        nc.sync.dma_start(
                            wrun[:], wsh[off:off + rl * CH_ELS].rearrange(
                                "(p f) -> p f", p=P))
                        qrun = outpool.tile([P, rl * OBW], BF16, tag="outr",
                                            name=f"qr{li}_{hh}")
                        for k in range(rl):
                            uu = upool.tile([P, OBW], F32, tag="u",
                                            name=f"uq{li}_{hh}_{k}")
                            nc.scalar.activation(uu[:],
                                                 wrun[:, k * OBW:(k + 1) * OBW],
                                                 AF.Copy, bias=MAGIC, scale=swl[:])
                            vv = sgpool.tile([P, OBW], F32, tag="sg",
                                             name=f"vq{li}_{hh}_{k}")
                            nc.vector.tensor_scalar(vv[:], uu[:], MAGIC, 1.0,
                                                    OP.subtract, OP.min)
                            nc.vector.tensor_scalar(qrun[:, k * OBW:(k + 1) * OBW],
                                                    vv[:], -1.0, None, OP.max)
                        soff = h * CH_ELS
                        nc.sync.dma_start(
                            stage[li][half][soff:soff + rl * CH_ELS].rearrange(
                                "(p f) -> p f", p=P),
                            qrun[:])
                    nc.gpsimd.collective_compute(
                        "AllGather", OP.bypass,
                        ins=[stage[li][half].opt()],
                        outs=[image[li][half].opt()],
                        replica_groups=[list(range(N_CORES))])

            def _scale_post(li, sumcol_ap):
                """From the AllReduced per-partition |W| sums: mean, 1/mean,
                partition-broadcast tiles."""
                L = layers[li]
                mean1 = small.tile([1, 1], F32, tag=f"mean{li}",
                                   name=f"mean{li}")
                nc.gpsimd.tensor_reduce(mean1[:], sumcol_ap,
                                        mybir.AxisListType.C, OP.add)
                nc.vector.tensor_scalar(mean1[:], mean1[:], 1.0 / L["numel"],
                                        float(EPS), OP.mult, OP.max)
                sw1 = small.tile([1, 1], F32, tag=f"sw{li}", name=f"sw{li}")
                nc.vector.reciprocal(sw1[:], mean1[:])
                mwl = small.tile([P, 1], F32, tag=f"mwb{li}", name=f"mwb{li}")
                swl = small.tile([P, 1], F32, tag=f"swb{li}", name=f"swb{li}")
                nc.gpsimd.partition_broadcast(mwl[:], mean1[:])
                nc.gpsimd.partition_broadcast(swl[:], sw1[:])
                return mwl, swl

            n_runs_total = sum((L["per_rank"] + RUN - 1) // RUN for L in layers)
            partials = small.tile([P, n_runs_total], F32, tag="partials",
                                  name="partials")
            mwb = [None] * 4

            # --- L0 chain (short; unblocks the main pass) ---
            nr0 = _abs_pass(0, 0, partials)
            pm0 = small.tile([P, 1], F32, tag="pm0", name="pm0")
            nc.vector.tensor_reduce(pm0[:], partials[:, 0:nr0],
                                    mybir.AxisListType.X, OP.add)
            nc.sync.dma_start(ar_in[0][:], pm0[:])
            nc.gpsimd.collective_compute(
                "AllReduce", OP.add,
                ins=[ar_in[0].opt()], outs=[ar_out[0].opt()],
                replica_groups=[list(range(N_CORES))])
            pms0 = small.tile([P, 1], F32, tag="pms0", name="pms0")
            nc.sync.dma_start(pms0[:], ar_out[0][:])
            mwb[0], swl0 = _scale_post(0, pms0[:])
            _quant_layer(0, swl0)

            # --- L1-3: one partial pass + one AllReduce ---
            col = nr0
            cols = {}
            for li in (1, 2, 3):
                cols[li] = col
                col += _abs_pass(li, col, partials)
            pm123 = small.tile([P, 3], F32, tag="pm123", name="pm123")
            for i, li in enumerate((1, 2, 3)):
                hi = col if li == 3 else cols[li + 1]
                nc.vector.tensor_reduce(pm123[:, i:i + 1],
                                        partials[:, cols[li]:hi],
                                        mybir.AxisListType.X, OP.add)
            nc.sync.dma_start(ar_in123[:], pm123[:])
            nc.gpsimd.collective_compute(
                "AllReduce", OP.add,
                ins=[ar_in123.opt()], outs=[ar_out123.opt()],
                replica_groups=[list(range(N_CORES))])
            pms123 = small.tile([P, 3], F32, tag="pms123", name="pms123")
            nc.sync.dma_start(pms123[:], ar_out123[:])
            for i, li in enumerate((1, 2, 3)):
                mwb[li], swl = _scale_post(li, pms123[:, i:i + 1])
                _quant_layer(li, swl)

            # per-row dequant scale for layer 0
            c_cur = []
            for t in range(T):
                c0 = small.tile([P, 1], F32, tag=f"c0_{t}", name=f"c0_{t}")
                nc.vector.scalar_tensor_tensor(c0[:], am0s[t][:], 1.0 / 127.0,
                                               mwb[0][:], OP.mult, OP.mult)
                c_cur.append(c0)

            # ---------------- Stage D: main pass ----------------
            for L in layers:
                li, n_ic, n_ob = L["li"], L["n_ic"], L["n_ob"]
                panel_ic, n_panels = L["panel_ic"], L["n_panels"]
                dout, dreal = L["dout"], L["dreal"]
                is_last = (li == 3)

                ys = [ypool.tile([P, dreal], F32, tag="y", name=f"y{li}_{t}")
                      for t in range(T)]
                bns = [small.tile([P, n_ob * 6], F32, tag=f"bn{t}",
                                  name=f"bn{li}_{t}")
                       for t in range(T)] if not is_last else None

                for ob in range(n_ob):
                    ow = L["ob_w"][ob]
                    ps = [psum.tile([P, OBW], F32, tag="ps",
                                    name=f"ps{li}_{ob}_{t}") for t in range(T)]
                    for panel in range(n_panels):
                        wp = wpool.tile([P, panel_ic, OBW], BF16, tag="wp",
                                        name=f"wp{li}_{ob}_{panel}")
                        g0 = (ob * n_ic + panel * panel_ic)  # global chunk
                        pr_l = L["per_rank"]
                        prh_l = pr_l // L["n_halves"]
                        rnk, j = divmod(g0, pr_l)
                        half, jl = divmod(j, prh_l)
                        uoff = (rnk * prh_l + jl) * CH_ELS
                        nc.sync.dma_start(
                            wp[:], image[li][half][uoff:uoff + panel_ic * CH_ELS]
                            .rearrange("(p c f) -> p c f", p=P, c=panel_ic))
                        for t in range(T):
                            for cc in range(panel_ic):
                                c = panel * panel_ic + cc
                                nc.tensor.matmul(
                                    ps[t][:], xqT_cur[t][:, c, :],
                                    wp[:, cc, :],
                                    start=(c == 0), stop=(c == n_ic - 1))
                    for t in range(T):
                        dst = ys[t][:, ob * OBW:ob * OBW + ow]
                        if not is_last:
                            nc.scalar.activation(dst, ps[t][:, :ow], AF.Copy,
                                                 bias=0.0, scale=c_cur[t][:])
                            nc.vector.bn_stats(bns[t][:, ob * 6:(ob + 1) * 6], dst)
                        else:
                            nc.scalar.activation(dst, ps[t][:, :ow], AF.Sigmoid,
                                                 bias=0.0, scale=c_cur[t][:])

                if is_last:
                    for t in range(T):
                        mzt = outpool.tile([P, OBINS], F32, tag="outr",
                                           name=f"mzt{t}")
                        nc.vector.tensor_scalar(mzt[:], ys[t][:, 0:OBINS],
                                                float(OBINS - 1), 1.0,
                                                OP.mult, OP.add)
                        nc.scalar.dma_start(mz_out[t * P:(t + 1) * P, :], mzt[:])
                        iit = outpool.tile([P, OBINS], F32, tag="outr",
                                           name=f"iit{t}")
                        nc.vector.tensor_scalar(iit[:], ys[t][:, OBINS:2 * OBINS],
                                                100.0, None, OP.mult)
                        nc.scalar.dma_start(ii_out[t * P:(t + 1) * P, :], iit[:])
                    continue

                # ---- tail: LN + SiLU + act quant + transpose ----
                n_ic_next = layers[li + 1]["n_ic"]
                xqT_next = []
                c_next = []
                for t in range(T):
                    mv = small.tile([P, 2], F32, tag="mv", name=f"mv{li}_{t}")
                    nc.vector.bn_aggr(mv[:], bns[t][:])
                    v = small.tile([P, 1], F32, tag="vvar", name=f"v{li}_{t}")
                    nc.vector.tensor_scalar(v[:], mv[:, 1:2], float(EPS), None,
                                            OP.add)
                    istd = _rsqrt_newton(nc, small, v[:])
                    nmi = small.tile([P, 1], F32, tag="nmi", name=f"nmi{li}_{t}")
                    nc.vector.scalar_tensor_tensor(nmi[:], mv[:, 0:1], -1.0,
                                                   istd[:], OP.mult, OP.mult)
                    # z = (y - mu) * istd, in place
                    nc.scalar.activation(ys[t][:], ys[t][:], AF.Identity,
                                         bias=nmi[:], scale=istd[:])
                    amsl = small.tile([P, 8], F32, tag="amsl",
                                      name=f"amsl{li}_{t}")
                    n_chk = dout // OBW
                    for ch in range(n_chk):
                        sl = ys[t][:, ch * OBW:(ch + 1) * OBW]
                        # sigmoid(z) = 0.5*tanh(0.5*z) + 0.5 (tanh table: 4 ULP)
                        sg = sgpool.tile([P, OBW], F32, tag="sg",
                                         name=f"sg{li}_{t}_{ch}")
                        nc.scalar.activation(sg[:], sl, AF.Tanh,
                                             bias=0.0, scale=0.5)
                        nc.vector.tensor_scalar(sg[:], sg[:], 0.5, 0.5,
                                                OP.mult, OP.add)
                        nc.vector.tensor_tensor(sl, sl, sg[:], OP.mult)
                        nc.vector.tensor_reduce(amsl[:, ch:ch + 1], sl,
                                                mybir.AxisListType.X, OP.max,
                                                apply_absolute_value=True)
                    am = small.tile([P, 1], F32, tag="amn", name=f"am{li}_{t}")
                    nc.vector.tensor_reduce(am[:], amsl[:, :n_chk],
                                            mybir.AxisListType.X, OP.max)
                    nc.vector.tensor_scalar(am[:], am[:], float(EPS), None,
                                            OP.max)
                    sc = small.tile([P, 1], F32, tag="scn", name=f"sc{li}_{t}")
                    nc.vector.tensor_scalar(sc[:], am[:], 1.0 / 127.0, None,
                                            OP.mult)
                    nc.vector.reciprocal(sc[:], sc[:])
                    cn = small.tile([P, 1], F32, tag=f"c{li + 1}_{t}",
                                    name=f"c{li + 1}_{t}")
                    nc.vector.scalar_tensor_tensor(cn[:], am[:], 1.0 / 127.0,
                                                   mwb[li + 1][:],
                                                   OP.mult, OP.mult)
                    c_next.append(cn)
                    xqn = xqnpool.tile([P, dout], BF16, tag="xqn",
                                       name=f"xqn{li}_{t}")
                    for ch in range(n_chk):
                        uu = upool.tile([P, OBW], F32, tag="u",
                                        name=f"ur{li}_{t}_{ch}")
                        nc.scalar.activation(uu[:], ys[t][:, ch * OBW:(ch + 1) * OBW],
                                             AF.Copy, bias=MAGIC, scale=sc[:])
                        nc.vector.tensor_scalar(xqn[:, ch * OBW:(ch + 1) * OBW],
                                                uu[:], MAGIC, None, OP.subtract)
                    xT = xqTpool.tile([P, n_ic_next, P], BF16, tag="xqT",
                                      name=f"xT{li}_{t}")
                    nc.scalar.dma_start_transpose(xT[:], xqn[:])
                    xqT_next.append(xT)
                xqT_cur = xqT_next
                c_cur = c_next

    nc.compile()
    return nc


def prepare_inputs(cfg, x, W0, W1, W2, W3):
    """Host-side sharding: per-core input maps. Weight chunks are shipped
    unit-major: unit u = (layer, ob, panel) is a [128, panel_ic*512] block,
    rows = partitions, contiguous per row; chunk cc of the unit holds
    W_l[ob*512+o, (panel*panel_ic+cc)*128+p] at [p, cc*512+o] (i.e. W^T)."""
    layers, total_ch, per_rank, T = _plan(cfg)
    b_core = T * P
    Ws = [np.asarray(W0), np.asarray(W1), np.asarray(W2), np.asarray(W3)]
    WTs = []
    for L, W in zip(layers, Ws):
        WT = np.zeros((L["din"], L["dout"]), dtype=np.float32)
        WT[:, :L["dreal"]] = W.T
        WTs.append(WT)

    shards = [np.empty(per_rank * CH_ELS, dtype=np.float32)
              for _ in range(N_CORES)]
    for L in layers:
        li, pr = L["li"], L["per_rank"]
        n_ic, panel_ic = L["n_ic"], L["panel_ic"]
        WT = WTs[li]
        for r in range(N_CORES):
            g0 = r * pr
            dst = shards[r]
            for j in range(0, pr, panel_ic):
                g = g0 + j
                ob, ic0 = divmod(g, n_ic)
                assert ic0 % panel_ic == 0
                # unit block [p, cc, o]
                blk = WT[ic0 * P:(ic0 + panel_ic) * P,
                         ob * OBW:(ob + 1) * OBW]          # [panel_ic*128, 512]
                blk = blk.reshape(panel_ic, P, OBW).transpose(1, 0, 2)
                off = (L["ch_base"] // N_CORES + j) * CH_ELS
                dst[off:off + panel_ic * CH_ELS] = blk.reshape(-1)
    x = np.asarray(x, dtype=np.float32)
    in_maps = []
    for r in range(N_CORES):
        in_maps.append(dict(
            xs=np.ascontiguousarray(x[r * b_core:(r + 1) * b_core]),
            wsh=shards[r],
        ))
    return in_maps


_NC_CACHE = {}


def _get_nc(cfg_key):
    if cfg_key not in _NC_CACHE:
        _NC_CACHE[cfg_key] = build(dict(cfg_key))
    return _NC_CACHE[cfg_key]


def run(cfg, x, W0, W1, W2, W3, trace=False):
    layers, total_ch, per_rank, T = _plan(cfg)
    b_core = T * P
    nc = _get_nc(tuple(sorted(cfg.items())))
    in_maps = prepare_inputs(cfg, x, W0, W1, W2, W3)
    res = run_bass_kernel_spmd(nc, in_maps, core_ids=list(range(N_CORES)),
                               trace=trace)
    mz = np.concatenate([res.results[r]["mz"] for r in range(N_CORES)], axis=0)
    ii = np.concatenate([res.results[r]["ii"] for r in range(N_CORES)], axis=0)
    return (mz, ii), res


def kernel(x, W0, W1, W2, W3, g0, b0, g1, b1, g2, b2):
    """Full-input entry point. g/b are identity (ones/zeros) in this problem's
    setup; LayerNorm affine is a no-op and is validated here."""
    for g in (g0, g1, g2):
        assert np.allclose(np.asarray(g), 1.0), "non-identity LN gain unsupported"
    for b in (b0, b1, b2):
        assert np.allclose(np.asarray(b), 0.0), "non-zero LN bias unsupported"
    (mz, ii), _ = run(FULL_CFG, x, W0, W1, W2, W3, trace=False)
    return (mz, ii)



# revision 3
# speedup vs baseline: 1.2826x; 1.2826x over previous
"""BitNet decoder MLP on 8 Trainium2 NeuronCores (Bass/Tile).

Strategy: data-parallel over batch (512 rows/core). Weights are ternary-quantized
cooperatively (each core quantizes 1/8 of all weight chunks; per-layer
AllReduce for the |W| mean, per-layer AllGather of the quantized image so
layer 0's weights are available early). The quantized image is fp8e4 (E4M3):
ternary {-1,0,1} is exact in fp8, and the tensor engine accepts mixed-dtype
matmuls (bf16 stationary activations x fp8 moving weights) at full rate, so
the image is half the bytes of bf16 with identical matmul time. All matmul
arithmetic is exact: activations are int8-valued bf16, weights {-1,0,1} fp8,
accumulation fp32 in PSUM. PSUM eviction + LayerNorm + SiLU are fused into
scalar-engine passes (dequant scale folds into the eviction; SiLU via the
hardware Silu table); rsqrt via Newton on the vector engine; rounding via the
fp32 magic-number trick (round-half-even, matches jnp.round).

Weight images are stored unit-major ("unit" = one weight panel, a
[128, panel_ic*512] block with contiguous per-partition rows) so every big DMA
moves 8KB-contiguous per-partition descriptors.
"""

import numpy as np

import concourse.bass as bass
import concourse.mybir as mybir
import concourse.tile as tile
from concourse import bacc
from concourse.bass_utils import run_bass_kernel_spmd

F32 = mybir.dt.float32
BF16 = mybir.dt.bfloat16
FP8 = mybir.dt.float8e4
AF = mybir.ActivationFunctionType
OP = mybir.AluOpType

N_CORES = 8
P = 128
OBW = 512            # output block width (one PSUM bank of fp32)
CH_ELS = P * OBW     # elements per weight chunk
MAGIC = 12582912.0   # 1.5 * 2**23: fp32 round-to-nearest-even trick
EPS = 1e-5

FULL_CFG = dict(B=4096, D0=1024, H=4096, OBINS=1000)


def _plan(cfg):
    """Static per-layer plan."""
    B, D0, H, OBINS = cfg["B"], cfg["D0"], cfg["H"], cfg["OBINS"]
    o3_real = 2 * OBINS
    o3_pad = ((o3_real + OBW - 1) // OBW) * OBW
    dims = [
        dict(din=D0, dout=H, dreal=H),
        dict(din=H, dout=H, dreal=H),
        dict(din=H, dout=H, dreal=H),
        dict(din=H, dout=o3_pad, dreal=o3_real),
    ]
    numels = [H * D0, H * H, H * H, o3_real * H]  # real numels for mean|W|
    layers = []
    ch_base = 0
    for li, d in enumerate(dims):
        n_ic = d["din"] // P
        n_ob = d["dout"] // OBW
        n_ch = n_ob * n_ic
        assert n_ch % N_CORES == 0, (li, n_ch)
        panel_ic = min(16, n_ic, max(1, n_ch // N_CORES))
        assert n_ic % panel_ic == 0 and (n_ch // N_CORES) % panel_ic == 0
        n_panels = n_ic // panel_ic
        layers.append(dict(
            li=li, din=d["din"], dout=d["dout"], dreal=d["dreal"],
            n_ic=n_ic, n_ob=n_ob, n_ch=n_ch, per_rank=n_ch // N_CORES,
            panel_ic=panel_ic, n_panels=n_panels,
            numel=numels[li], ch_base=ch_base,
            n_halves=2 if (n_ch // N_CORES) % 2 == 0 and (n_ch // N_CORES) // 2 % panel_ic == 0 else 1,
            ob_w=[min(OBW, d["dreal"] - ob * OBW) for ob in range(n_ob)],
        ))
        ch_base += n_ch
    total_ch = ch_base
    per_rank = total_ch // N_CORES
    b_core = B // N_CORES
    assert b_core % P == 0
    return layers, total_ch, per_rank, b_core // P


def _rsqrt_newton(nc, pool, v, n_iter=3):
    """istd = 1/sqrt(v) for v [128,1] fp32 (v > 0), pure-DVE Newton iteration.

    seed_bits = 0x5f370000 - bits(v)/2 computed in fp32 on aligned int32 views;
    the fp32 mantissa noise on the >2^24 intermediate is irrelevant for a seed.
    """
    seed = pool.tile([P, 1], F32, tag="rs_seed", name="rs_seed")
    seed_i32 = seed[:].bitcast(mybir.dt.int32)
    v_i32 = v.bitcast(mybir.dt.int32)
    nc.vector.tensor_scalar(seed_i32[:], v_i32[:], -0.5,
                            float(0x5F370000), OP.mult, OP.add)
    y = seed
    t1 = pool.tile([P, 1], F32, tag="rs_t1", name="rs_t1")
    t2 = pool.tile([P, 1], F32, tag="rs_t2", name="rs_t2")
    for _ in range(n_iter):
        nc.vector.tensor_tensor(t1[:], y[:], y[:], OP.mult)
        nc.vector.tensor_tensor(t2[:], t1[:], v, OP.mult)
        nc.vector.tensor_scalar(t1[:], t2[:], -0.5, 1.5, OP.mult, OP.add)
        nc.vector.tensor_tensor(y[:], y[:], t1[:], OP.mult)
    return y


def build(cfg):
    layers, total_ch, per_rank, T = _plan(cfg)
    nc = bacc.Bacc("TRN2", target_bir_lowering=False, debug=False,
                   num_devices=N_CORES)

    D0, OBINS = cfg["D0"], cfg["OBINS"]
    b_core = T * P

    xs = nc.dram_tensor("xs", [b_core, D0], F32, kind="ExternalInput")
    # rank's weight chunks, unit-major flat fp32 (see prepare_inputs)
    wsh = nc.dram_tensor("wsh", [per_rank * CH_ELS], F32, kind="ExternalInput")
    mz_out = nc.dram_tensor("mz", [b_core, OBINS], F32, kind="ExternalOutput")
    ii_out = nc.dram_tensor("ii", [b_core, OBINS], F32, kind="ExternalOutput")

    with tile.TileContext(nc) as tc:
        with (
            tc.tile_pool(name="ybig", bufs=4) as ypool,        # 16KB/partition f32
            tc.tile_pool(name="wr", bufs=3) as wrpool,         # prep fp32 runs 8KB
            tc.tile_pool(name="xqT", bufs=5) as xqTpool,       # [128,32,128] bf16
            tc.tile_pool(name="xqT0", bufs=4) as xqT0pool,     # [128,n_ic0,128] bf16
            tc.tile_pool(name="wp", bufs=3) as wpool,          # [128,16,512] fp8
            tc.tile_pool(name="xqn", bufs=2) as xqnpool,       # 8KB/partition bf16
            tc.tile_pool(name="u", bufs=2) as upool,           # [128,2048] f32
            tc.tile_pool(name="q8", bufs=2) as qpool,          # [128,2048] fp8
            tc.tile_pool(name="outr", bufs=2) as outpool,      # [128,OBINS] f32
            tc.tile_pool(name="small", bufs=1) as small,
            tc.tile_pool(name="psum", bufs=8, space="PSUM") as psum,
            tc.tile_pool(name="dram", bufs=1, space="DRAM") as dram,
        ):
            # ---------------- DRAM scratch (flat, unit-major, fp8) -----------
            stage = []
            image = []
            for L in layers:
                nh = L["n_halves"]
                hs = L["per_rank"] // nh * CH_ELS
                hi = L["n_ch"] // nh * CH_ELS
                stage.append([dram.tile([hs], FP8, tag=f"stage{L['li']}_{h}",
                                        name=f"stage{L['li']}_{h}")
                              for h in range(nh)])
                image.append([dram.tile([hi], FP8, tag=f"image{L['li']}_{h}",
                                        name=f"image{L['li']}_{h}",
                                        addr_space="Shared")
                              for h in range(nh)])
            ar_in = [dram.tile([P, 1], F32, tag=f"ar_in{l}", name=f"ar_in{l}")
                     for l in range(2)]
            ar_out = [dram.tile([P, 1], F32, tag=f"ar_out{l}",
                                name=f"ar_out{l}", addr_space="Shared")
                      for l in range(2)]
            ar_in23 = dram.tile([P, 2], F32, tag="ar_in23", name="ar_in23")
            ar_out23 = dram.tile([P, 2], F32, tag="ar_out23",
                                 name="ar_out23", addr_space="Shared")

            RUN = 4
            layer_jofs = {}
            jofs = 0
            for L in layers:
                layer_jofs[L["li"]] = jofs
                jofs += L["per_rank"]

            n_runs_total = sum((L["per_rank"] + RUN - 1) // RUN for L in layers)
            partials = small.tile([P, n_runs_total], F32, tag="partials",
                                  name="partials")
            mwb = [None] * 4

            def _abs_pass(li, partial_col, partials):
                L = layers[li]
                pr, jofs = L["per_rank"], layer_jofs[li]
                nrun = 0
                for h in range(0, pr, RUN):
                    rl = min(RUN, pr - h)
                    off = (jofs + h) * CH_ELS
                    wrun = wrpool.tile([P, rl * OBW], F32, tag="wr",
                                       name=f"wrB{li}_{h}")
                    nc.sync.dma_start(
                        wrun[:], wsh[off:off + rl * CH_ELS].rearrange(
                            "(p f) -> p f", p=P))
                    nc.scalar.activation(wrun[:], wrun[:], AF.Abs,
                                         bias=0.0, scale=1.0,
                                         accum_out=partials[:, partial_col + nrun:
                                                            partial_col + nrun + 1])
                    nrun += 1
                return nrun

            def _quant_layer(li, swl):
                L = layers[li]
                pr, jofs = L["per_rank"], layer_jofs[li]
                nh = L["n_halves"]
                prh = pr // nh
                for half in range(nh):
                    for h in range(0, prh, RUN):
                        rl = min(RUN, prh - h)
                        hh = half * prh + h
                        off = (jofs + hh) * CH_ELS
                        wrun = wrpool.tile([P, rl * OBW], F32, tag="wr",
                                           name=f"wrC{li}_{hh}")
                        nc.sync.dma_start(
                            wrun[:], wsh[off:off + rl * CH_ELS].rearrange(
                                "(p f) -> p f", p=P))
                        # round(w*s) via magic, clip to [-1,1], write fp8
                        uu = upool.tile([P, rl * OBW], F32, tag="u",
                                        name=f"uq{li}_{hh}")
                        nc.scalar.activation(uu[:], wrun[:], AF.Copy,
                                             bias=MAGIC, scale=swl[:])
                        nc.vector.tensor_scalar(uu[:], uu[:], MAGIC, 1.0,
                                                OP.subtract, OP.min)
                        qrun = qpool.tile([P, rl * OBW], FP8, tag="q8",
                                          name=f"qr{li}_{hh}")
                        nc.vector.tensor_scalar(qrun[:], uu[:], -1.0, None,
                                                OP.max)
                        soff = h * CH_ELS
                        nc.sync.dma_start(
                            stage[li][half][soff:soff + rl * CH_ELS].rearrange(
                                "(p f) -> p f", p=P),
                            qrun[:])
                    nc.gpsimd.collective_compute(
                        "AllGather", OP.bypass,
                        ins=[stage[li][half].opt()],
                        outs=[image[li][half].opt()],
                        replica_groups=[list(range(N_CORES))])

            def _scale_post(li, sumcol_ap):
                """From the AllReduced per-partition |W| sums: mean|W| and
                1/mean|W| broadcast to all partitions."""
                L = layers[li]
                import concourse.bass_isa as bass_isa
                mean_all = small.tile([P, 1], F32, tag=f"mean{li}",
                                      name=f"mean{li}")
                nc.gpsimd.partition_all_reduce(mean_all[:], sumcol_ap,
                                               channels=P,
                                               reduce_op=bass_isa.ReduceOp.add)
                mwl = small.tile([P, 1], F32, tag=f"mwb{li}", name=f"mwb{li}")
                nc.vector.tensor_scalar(mwl[:], mean_all[:], 1.0 / L["numel"],
                                        float(EPS), OP.mult, OP.max)
                swl = small.tile([P, 1], F32, tag=f"swb{li}", name=f"swb{li}")
                nc.vector.reciprocal(swl[:], mwl[:])
                return mwl, swl

            # ---- L0 abs chain first: unblocks AR1 + L0 quant ASAP ----
            nr0 = _abs_pass(0, 0, partials)
            pm0 = small.tile([P, 1], F32, tag="pm0", name="pm0")
            nc.vector.tensor_reduce(pm0[:], partials[:, 0:nr0],
                                    mybir.AxisListType.X, OP.add)
            nc.sync.dma_start(ar_in[0][:], pm0[:])
            nc.gpsimd.collective_compute(
                "AllReduce", OP.add,
                ins=[ar_in[0].opt()], outs=[ar_out[0].opt()],
                replica_groups=[list(range(N_CORES))])

            # ---------------- Stage A: input activation quant ----------------
            n_ic0 = layers[0]["n_ic"]
            xqT_cur = []
            am0s = []
            for t in range(T):
                xt = ypool.tile([P, D0], F32, tag="y", name=f"xt{t}")
                nc.sync.dma_start(xt[:], xs[t * P:(t + 1) * P, :])
                am = small.tile([P, 1], F32, tag=f"am0_{t}", name=f"am0_{t}")
                nc.vector.tensor_reduce(am[:], xt[:], mybir.AxisListType.X,
                                        OP.max, apply_absolute_value=True)
                nc.vector.tensor_scalar(am[:], am[:], float(EPS), None, OP.max)
                sc = small.tile([P, 1], F32, tag=f"s0_{t}", name=f"s0_{t}")
                nc.vector.tensor_scalar(sc[:], am[:], 1.0 / 127.0, None, OP.mult)
                nc.vector.reciprocal(sc[:], sc[:])
                xq0 = xqnpool.tile([P, D0], BF16, tag="xqn", name=f"xq0_{t}")
                uu = upool.tile([P, D0], F32, tag="u", name=f"u0_{t}")
                nc.scalar.activation(uu[:], xt[:], AF.Copy, bias=MAGIC,
                                     scale=sc[:])
                nc.vector.tensor_scalar(xq0[:], uu[:], MAGIC, None, OP.subtract)
                xqT0 = xqT0pool.tile([P, n_ic0, P], BF16, tag="xqT0",
                                     name=f"xqT0_{t}")
                nc.sync.dma_start_transpose(xqT0[:], xq0[:])
                xqT_cur.append(xqT0)
                am0s.append(am)

            # ---- L1 abs + AR2; finish L0 scale + quant + gather ----
            col = nr0
            cols = {}
            cols[1] = col
            col += _abs_pass(1, col, partials)
            pm1 = small.tile([P, 1], F32, tag="pm1", name="pm1")
            nc.vector.tensor_reduce(pm1[:], partials[:, cols[1]:col],
                                    mybir.AxisListType.X, OP.add)
            nc.sync.dma_start(ar_in[1][:], pm1[:])
            nc.gpsimd.collective_compute(
                "AllReduce", OP.add,
                ins=[ar_in[1].opt()], outs=[ar_out[1].opt()],
                replica_groups=[list(range(N_CORES))])

            pms0 = small.tile([P, 1], F32, tag="pms0", name="pms0")
            nc.sync.dma_start(pms0[:], ar_out[0][:])
            mwb[0], swl0 = _scale_post(0, pms0[:])
            _quant_layer(0, swl0)

            # ---- L2+L3 abs + AR3; finish L1 ----
            for li in (2, 3):
                cols[li] = col
                col += _abs_pass(li, col, partials)
            pm23 = small.tile([P, 2], F32, tag="pm23", name="pm23")
            for i, li in enumerate((2, 3)):
                hi = col if li == 3 else cols[3]
                nc.vector.tensor_reduce(pm23[:, i:i + 1],
                                        partials[:, cols[li]:hi],
                                        mybir.AxisListType.X, OP.add)
            nc.sync.dma_start(ar_in23[:], pm23[:])
            nc.gpsimd.collective_compute(
                "AllReduce", OP.add,
                ins=[ar_in23.opt()], outs=[ar_out23.opt()],
                replica_groups=[list(range(N_CORES))])

            pms1 = small.tile([P, 1], F32, tag="pms1", name="pms1")
            nc.sync.dma_start(pms1[:], ar_out[1][:])
            mwb[1], swl1 = _scale_post(1, pms1[:])
            _quant_layer(1, swl1)

            pms23 = small.tile([P, 2], F32, tag="pms23", name="pms23")
            nc.sync.dma_start(pms23[:], ar_out23[:])
            for i, li in enumerate((2, 3)):
                mwb[li], swl = _scale_post(li, pms23[:, i:i + 1])
                _quant_layer(li, swl)

            # per-row dequant scale for layer 0
            c_cur = []
            for t in range(T):
                c0 = small.tile([P, 1], F32, tag=f"c0_{t}", name=f"c0_{t}")
                nc.vector.scalar_tensor_tensor(c0[:], am0s[t][:], 1.0 / 127.0,
                                               mwb[0][:], OP.mult, OP.mult)
                c_cur.append(c0)

            # ---------------- Stage D: main pass ----------------
            for L in layers:
                li, n_ic, n_ob = L["li"], L["n_ic"], L["n_ob"]
                panel_ic, n_panels = L["panel_ic"], L["n_panels"]
                dout, dreal = L["dout"], L["dreal"]
                is_last = (li == 3)

                ys = [ypool.tile([P, dreal], F32, tag="y", name=f"y{li}_{t}")
                      for t in range(T)]
                bns = [small.tile([P, n_ob * 6], F32, tag=f"bn{t}",
                                  name=f"bn{li}_{t}")
                       for t in range(T)] if not is_last else None

                for ob in range(n_ob):
                    ow = L["ob_w"][ob]
                    ps = [psum.tile([P, OBW], F32, tag="ps",
                                    name=f"ps{li}_{ob}_{t}") for t in range(T)]
                    for panel in range(n_panels):
                        wp = wpool.tile([P, panel_ic, OBW], FP8, tag="wp",
                                        name=f"wp{li}_{ob}_{panel}")
                        g0 = (ob * n_ic + panel * panel_ic)  # global chunk
                        pr_l = L["per_rank"]
                        prh_l = pr_l // L["n_halves"]
                        rnk, j = divmod(g0, pr_l)
                        half, jl = divmod(j, prh_l)
                        uoff = (rnk * prh_l + jl) * CH_ELS
                        nc.sync.dma_start(
                            wp[:], image[li][half][uoff:uoff + panel_ic * CH_ELS]
                            .rearrange("(p c f) -> p c f", p=P, c=panel_ic))
                        for t in range(T):
                            for cc in range(panel_ic):
                                c = panel * panel_ic + cc
                                nc.tensor.matmul(
                                    ps[t][:], xqT_cur[t][:, c, :],
                                    wp[:, cc, :],
                                    start=(c == 0), stop=(c == n_ic - 1))
                    for t in range(T):
                        dst = ys[t][:, ob * OBW:ob * OBW + ow]
                        if not is_last:
                            nc.scalar.activation(dst, ps[t][:, :ow], AF.Copy,
                                                 bias=0.0, scale=c_cur[t][:])
                            nc.vector.bn_stats(bns[t][:, ob * 6:(ob + 1) * 6], dst)
                        else:
                            nc.scalar.activation(dst, ps[t][:, :ow], AF.Sigmoid,
                                                 bias=0.0, scale=c_cur[t][:])

                if is_last:
                    for t in range(T):
                        mzt = outpool.tile([P, OBINS], F32, tag="outr",
                                           name=f"mzt{t}")
                        nc.vector.tensor_scalar(mzt[:], ys[t][:, 0:OBINS],
                                                float(OBINS - 1), 1.0,
                                                OP.mult, OP.add)
                        nc.scalar.dma_start(mz_out[t * P:(t + 1) * P, :], mzt[:])
                        iit = outpool.tile([P, OBINS], F32, tag="outr",
                                           name=f"iit{t}")
                        nc.vector.tensor_scalar(iit[:], ys[t][:, OBINS:2 * OBINS],
                                                100.0, None, OP.mult)
                        nc.scalar.dma_start(ii_out[t * P:(t + 1) * P, :], iit[:])
                    continue

                # ---- tail: fused LN+SiLU, act quant, transpose ----
                n_ic_next = layers[li + 1]["n_ic"]
                QW = min(2048, dout)
                xqT_next = []
                c_next = []
                for t in range(T):
                    mv = small.tile([P, 2], F32, tag="mv", name=f"mv{li}_{t}")
                    nc.vector.bn_aggr(mv[:], bns[t][:])
                    v = small.tile([P, 1], F32, tag="vvar", name=f"v{li}_{t}")
                    nc.vector.tensor_scalar(v[:], mv[:, 1:2], float(EPS), None,
                                            OP.add)
                    istd = _rsqrt_newton(nc, small, v[:])
                    nmi = small.tile([P, 1], F32, tag="nmi", name=f"nmi{li}_{t}")
                    nc.vector.scalar_tensor_tensor(nmi[:], mv[:, 0:1], -1.0,
                                                   istd[:], OP.mult, OP.mult)
                    # h = silu((y - mu) * istd), in place (hw Silu table)
                    nc.scalar.activation(ys[t][:], ys[t][:], AF.Silu,
                                         bias=nmi[:], scale=istd[:])
                    am = small.tile([P, 1], F32, tag="amn", name=f"am{li}_{t}")
                    nc.vector.tensor_reduce(am[:], ys[t][:],
                                            mybir.AxisListType.X, OP.max,
                                            apply_absolute_value=True)
                    nc.vector.tensor_scalar(am[:], am[:], float(EPS), None,
                                            OP.max)
                    sc = small.tile([P, 1], F32, tag="scn", name=f"sc{li}_{t}")
                    nc.vector.tensor_scalar(sc[:], am[:], 1.0 / 127.0, None,
                                            OP.mult)
                    nc.vector.reciprocal(sc[:], sc[:])
                    cn = small.tile([P, 1], F32, tag=f"c{li + 1}_{t}",
                                    name=f"c{li + 1}_{t}")
                    nc.vector.scalar_tensor_tensor(cn[:], am[:], 1.0 / 127.0,
                                                   mwb[li + 1][:],
                                                   OP.mult, OP.mult)
                    c_next.append(cn)
                    xqn = xqnpool.tile([P, dout], BF16, tag="xqn",
                                       name=f"xqn{li}_{t}")
                    for ch in range(dout // QW):
                        uu = upool.tile([P, QW], F32, tag="u",
                                        name=f"ur{li}_{t}_{ch}")
                        nc.scalar.activation(uu[:], ys[t][:, ch * QW:(ch + 1) * QW],
                                             AF.Copy, bias=MAGIC, scale=sc[:])
                        nc.vector.tensor_scalar(xqn[:, ch * QW:(ch + 1) * QW],
                                                uu[:], MAGIC, None, OP.subtract)
                    xT = xqTpool.tile([P, n_ic_next, P], BF16, tag="xqT",
                                      name=f"xT{li}_{t}")
                    nc.sync.dma_start_transpose(xT[:], xqn[:])
                    xqT_next.append(xT)
                xqT_cur = xqT_next
                c_cur = c_next

    nc.compile()
    return nc


def prepare_inputs(cfg, x, W0, W1, W2, W3):
    """Host-side sharding: per-core input maps. Weight chunks are shipped
    unit-major: unit u = (layer, ob, panel) is a [128, panel_ic*512] block,
    rows = partitions, contiguous per row; chunk cc of the unit holds
    W_l[ob*512+o, (panel*panel_ic+cc)*128+p] at [p, cc*512+o] (i.e. W^T)."""
    layers, total_ch, per_rank, T = _plan(cfg)
    b_core = T * P
    Ws = [np.asarray(W0), np.asarray(W1), np.asarray(W2), np.asarray(W3)]
    WTs = []
    for L, W in zip(layers, Ws):
        WT = np.zeros((L["din"], L["dout"]), dtype=np.float32)
        WT[:, :L["dreal"]] = W.T
        WTs.append(WT)

    shards = [np.empty(per_rank * CH_ELS, dtype=np.float32)
              for _ in range(N_CORES)]
    for L in layers:
        li, pr = L["li"], L["per_rank"]
        n_ic, panel_ic = L["n_ic"], L["panel_ic"]
        WT = WTs[li]
        for r in range(N_CORES):
            g0 = r * pr
            dst = shards[r]
            for j in range(0, pr, panel_ic):
                g = g0 + j
                ob, ic0 = divmod(g, n_ic)
                assert ic0 % panel_ic == 0
                # unit block [p, cc, o]
                blk = WT[ic0 * P:(ic0 + panel_ic) * P,
                         ob * OBW:(ob + 1) * OBW]          # [panel_ic*128, 512]
                blk = blk.reshape(panel_ic, P, OBW).transpose(1, 0, 2)
                off = (L["ch_base"] // N_CORES + j) * CH_ELS
                dst[off:off + panel_ic * CH_ELS] = blk.reshape(-1)
    x = np.asarray(x, dtype=np.float32)
    in_maps = []
    for r in range(N_CORES):
        in_maps.append(dict(
            xs=np.ascontiguousarray(x[r * b_core:(r + 1) * b_core]),
            wsh=shards[r],
        ))
    return in_maps


_NC_CACHE = {}


def _get_nc(cfg_key):
    if cfg_key not in _NC_CACHE:
        _NC_CACHE[cfg_key] = build(dict(cfg_key))
    return _NC_CACHE[cfg_key]


def run(cfg, x, W0, W1, W2, W3, trace=False):
    layers, total_ch, per_rank, T = _plan(cfg)
    b_core = T * P
    nc = _get_nc(tuple(sorted(cfg.items())))
    in_maps = prepare_inputs(cfg, x, W0, W1, W2, W3)
    res = run_bass_kernel_spmd(nc, in_maps, core_ids=list(range(N_CORES)),
                               trace=trace)
    mz = np.concatenate([res.results[r]["mz"] for r in range(N_CORES)], axis=0)
    ii = np.concatenate([res.results[r]["ii"] for r in range(N_CORES)], axis=0)
    return (mz, ii), res


def kernel(x, W0, W1, W2, W3, g0, b0, g1, b1, g2, b2):
    """Full-input entry point. g/b are identity (ones/zeros) in this problem's
    setup; LayerNorm affine is a no-op and is validated here."""
    for g in (g0, g1, g2):
        assert np.allclose(np.asarray(g), 1.0), "non-identity LN gain unsupported"
    for b in (b0, b1, b2):
        assert np.allclose(np.asarray(b), 0.0), "non-zero LN bias unsupported"
    (mz, ii), _ = run(FULL_CFG, x, W0, W1, W2, W3, trace=False)
    return (mz, ii)
